# revision 1
# baseline (speedup 1.0000x reference)
"""Tacotron2-style decoder on 8 TRN2 cores (Bass/Tile, SPMD).

Sharding: gate-dim model parallelism (8-way) for both LSTMs (weights stay
SBUF-resident); attention, alignments and memory batch-sharded (4 ex/core);
mel/gate projection replicated (core 0's output used).

Per step: 2 AllGathers — AG1 carries [dhT(t-1); ahT(t)] merged, AG2 carries
ctx(t). Gate rows permuted to [i,f,o,g] so sigmoid is one contiguous ACT op;
sigmoid computed as 0.5*tanh(0.5x)+0.5 so only the exp_and_others ACT table
set is used (tanh/exp/relu/copy — no per-step table switches).
"""
import numpy as np
import ml_dtypes
import bass_rust
import concourse.bass as bass
import concourse.bacc as bacc
import concourse.tile as tile
import concourse.mybir as mybir
import concourse.masks as masks

FP32 = mybir.dt.float32
BF16 = mybir.dt.bfloat16
AFT = mybir.ActivationFunctionType
AXL = mybir.AxisListType
ALU = mybir.AluOpType

B, T, E = 32, 512, 512
NMEL = 80
PRE = 256
ATT_DIM = 128
FILT, KS = 31, 31
PAD = (KS - 1) // 2
NC = 8
BL = B // NC       # 4 examples per core
G = 4096 // NC     # 512 gate slice
HL = 1024 // NC    # 128 hidden slice
TP = T + 2 * PAD   # 542


def strided(ap, dims, extra_offset=0):
    c = ap.copy()
    c.ap = bass_rust.VecI64Pair([list(d) for d in dims])
    c.offset = ap.offset + extra_offset
    return c


def gate_perm(core):
    idx = []
    for blk in (0, 1, 3, 2):  # i, f, o, g
        s = blk * 1024 + core * HL
        idx.extend(range(s, s + HL))
    return np.array(idx)


def host_inputs(inputs, td):
    f32 = lambda x: np.ascontiguousarray(np.asarray(x, dtype=np.float32))
    bf = lambda x: np.ascontiguousarray(np.asarray(x, dtype=np.float32).astype(ml_dtypes.bfloat16))
    Wih_att, Whh_att = f32(inputs["Wih_att"]), f32(inputs["Whh_att"])
    Wih_dec, Whh_dec = f32(inputs["Wih_dec"]), f32(inputs["Whh_dec"])
    batt = f32(inputs["bih_att"]) + f32(inputs["bhh_att"])
    bdec = f32(inputs["bih_dec"]) + f32(inputs["bhh_dec"])
    W_score, b_score = f32(inputs["W_score"]), f32(inputs["b_score"])
    memory = f32(inputs["memory"])
    mask = np.asarray(inputs["mask_seq"])

    frames = f32(inputs["decoder_inputs"]).transpose(2, 0, 1)  # [TD,B,80]
    pin = np.concatenate([np.zeros_like(frames[:1]), frames[:-1]], 0)[:td]
    pinT = f32(pin.reshape(td * B, NMEL).T)

    Wpg = np.concatenate([f32(inputs["W_proj"]), f32(inputs["W_gate"])], 0)
    bpg = np.concatenate([f32(inputs["b_proj"]), f32(inputs["b_gate"])],
                         0).reshape(1, 81)
    wconv_unf = f32(inputs["W_loc_conv"]).transpose(1, 2, 0).reshape(62, FILT)
    wloc_comb = np.ascontiguousarray(wconv_unf @ f32(inputs["W_loc_dense"]).T)

    wsc = np.zeros((ATT_DIM, 16), np.float32)
    for b in range(BL):
        wsc[:, b * 4 + b] = W_score[0]

    maps = []
    for c in range(NC):
        rows = gate_perm(c)
        ex = slice(c * BL, (c + 1) * BL)
        qsel = np.zeros((B, BL), np.float32)
        for b in range(BL):
            qsel[c * BL + b, b] = 1.0
        m = {
            "att_ctx_T": f32(Wih_att[rows, 256:768].T),
            "att_h_T": f32(Whh_att[rows, :].T),
            "att_pre_T": f32(Wih_att[rows, 0:256].T),
            "att_bias": f32(np.broadcast_to(batt[rows][None, :], (128, G)).copy()),
            "dec_ah_T": f32(Wih_dec[rows, 0:1024].T),
            "dec_ctx_T": f32(Wih_dec[rows, 1024:1536].T),
            "dec_h_T": f32(Whh_dec[rows, :].T),
            "dec_bias": f32(np.broadcast_to(bdec[rows][None, :], (B, G)).copy()),
            "wq_T": f32(inputs["W_query"]).T.copy(),
            "qsel": qsel,
            "wsc_mask": bf(wsc),
            "wloc_comb": bf(wloc_comb),
            "wpg_T": f32(Wpg.T),
            "pg_bias": f32(np.broadcast_to(bpg, (B, 81)).copy()),
            "w_mem_T": bf(f32(inputs["W_memory"]).T),
            "mem_in": bf(memory[ex]),
            "mask_bias": np.where(mask[ex], np.float32(-1e9),
                                  np.float32(b_score[0])).astype(np.float32),
            "pinT": pinT,
            "w_pre1_T": f32(inputs["W_pre1"]).T.copy(),
            "w_pre2_T": f32(inputs["W_pre2"]).T.copy(),
        }
        maps.append(m)
    return maps


def postprocess(results, td):
    """Assemble (mel, gate, align) from per-core outputs."""
    mg = results[0]["melgate_out"]          # [td, B, 81]
    mel = mg[:, :, :NMEL].transpose(1, 2, 0)        # [B, 80, td]
    gate = mg[:, :, NMEL].transpose(1, 0)           # [B, td]
    align = np.concatenate([r["align_out"] for r in results], 1)  # [td,B,T]
    align = align.transpose(1, 0, 2)                # [B, td, T]
    return (np.ascontiguousarray(mel), np.ascontiguousarray(gate),
            np.ascontiguousarray(align))


def build(td):
    nc = bacc.Bacc("TRN2", target_bir_lowering=False, debug=False,
                   num_devices=NC)
    R = td * B
    din = {}
    for name, shape in [
        ("att_ctx_T", [512, G]), ("att_h_T", [1024, G]), ("att_pre_T", [256, G]),
        ("att_bias", [128, G]), ("dec_ah_T", [1024, G]), ("dec_ctx_T", [512, G]),
        ("dec_h_T", [1024, G]), ("dec_bias", [B, G]), ("wq_T", [1024, 128]),
        ("qsel", [B, BL]),  ("wpg_T", [1536, 81]), ("pg_bias", [B, 81]),
        ("mask_bias", [BL, T]),
        ("pinT", [NMEL, R]), ("w_pre1_T", [NMEL, PRE]), ("w_pre2_T", [PRE, PRE]),
    ]:
        din[name] = nc.dram_tensor(name, shape, FP32, kind="ExternalInput")
    for name, shape in [
        ("wsc_mask", [128, 16]), ("wloc_comb", [62, 128]),
        ("w_mem_T", [512, 128]), ("mem_in", [BL, T, E]),
    ]:
        din[name] = nc.dram_tensor(name, shape, mybir.dt.bfloat16,
                                   kind="ExternalInput")

    melgate_out = nc.dram_tensor("melgate_out", [td, B, 81], FP32,
                                 kind="ExternalOutput")
    align_out = nc.dram_tensor("align_out", [td, BL, T], FP32,
                               kind="ExternalOutput")

    with tile.TileContext(nc) as tc:
        _body(nc, tc, din, melgate_out, align_out, td)
    nc.compile()
    return nc


def _body(nc, tc, din, melgate_out, align_out, td):
    R = td * B
    rb_sizes = []
    off = 0
    while off < R:
        rb_sizes.append(min(512, R - off))
        off += 512

    import contextlib
    stack = contextlib.ExitStack()
    st = stack.enter_context(tc.tile_pool(name="state", bufs=1))
    dr = stack.enter_context(tc.tile_pool(name="dram", bufs=1, space="DRAM"))
    pg_pool = stack.enter_context(tc.tile_pool(name="pgates", bufs=2, space="PSUM"))
    pat_pool = stack.enter_context(tc.tile_pool(name="pattn", bufs=2, space="PSUM"))
    pt_pool = stack.enter_context(tc.tile_pool(name="ptrans", bufs=2, space="PSUM"))
    ps1_pool = stack.enter_context(tc.tile_pool(name="psmall", bufs=2, space="PSUM"))
    sb2 = stack.enter_context(tc.tile_pool(name="sb2", bufs=1))
    sb1 = stack.enter_context(tc.tile_pool(name="sb1", bufs=1))
    sb3 = stack.enter_context(tc.tile_pool(name="sb3", bufs=2))
    drc = stack.enter_context(tc.tile_pool(name="drc", bufs=3, space="DRAM"))

    ident = st.tile([128, 128], FP32)
    masks.make_identity(nc, ident[:])
    ident_bf = st.tile([128, 128], BF16)
    masks.make_identity(nc, ident_bf[:])

    def load_kmaj(name, kdim, ndim, dt=FP32):
        t = st.tile([128, (kdim // 128) * ndim], dt, name=f"sb_{name}")
        dma = nc.sync.dma_start if dt == FP32 else nc.gpsimd.dma_start
        dma(t[:].rearrange("p (k n) -> p k n", k=kdim // 128),
            din[name].ap().rearrange("(k p) n -> p k n", p=128))
        return t

    w_att_ctx = load_kmaj("att_ctx_T", 512, G, BF16)
    w_att_h = load_kmaj("att_h_T", 1024, G, BF16)
    w_dec_ah = load_kmaj("dec_ah_T", 1024, G, BF16)
    w_dec_ctx = load_kmaj("dec_ctx_T", 512, G, BF16)
    w_dec_h = load_kmaj("dec_h_T", 1024, G, BF16)
    w_q = load_kmaj("wq_T", 1024, 128, BF16)
    w_pg = load_kmaj("wpg_T", 1536, 81, BF16)

    def load_plain(name, shape):
        t = st.tile(shape, FP32, name=f"sb_{name}")
        nc.sync.dma_start(t[:], din[name].ap())
        return t

    def load_bf16(name, shape):
        t = st.tile(shape, BF16, name=f"sb_{name}")
        nc.sync.dma_start(t[:], din[name].ap())
        return t

    wsc = load_bf16("wsc_mask", [128, 16])
    wloc = load_bf16("wloc_comb", [62, 128])
    pgb = load_plain("pg_bias", [B, 81])
    decb = load_plain("dec_bias", [B, G])
    mkb = load_plain("mask_bias", [BL, T])
    qsel = load_plain("qsel", [B, BL])

    mem = st.tile([128, BL * 4 * 512], BF16)
    nc.sync.dma_start(
        mem[:].rearrange("p (b k e) -> p b k e", b=BL, k=4),
        din["mem_in"].ap().rearrange("b (k p) e -> p b k e", p=128))

    # ---- prologue: pmT[A, (b,t)] = W_memory @ mem[b].T ----
    pmT = st.tile([128, BL * T], BF16)
    with tc.tile_pool(name="prol", bufs=2) as prol:
        w_mem = prol.tile([128, 4 * 128], BF16, name="w_mem", tag="w_mem",
                          bufs=1)
        nc.sync.dma_start(
            w_mem[:].rearrange("p (k n) -> p k n", k=4),
            din["w_mem_T"].ap().rearrange("(k p) n -> p k n", p=128))
        for b in range(BL):
            for tcn in range(4):
                pmp = ps1_pool.tile([128, 128], FP32, name="pmp", tag="ps1")
                for ec in range(4):
                    tp = pt_pool.tile([128, 128], BF16, name="tp", tag="tr")
                    base = (b * 4 + tcn) * 512 + ec * 128
                    nc.tensor.transpose(tp[:], mem[:, base:base + 128],
                                        ident_bf[:])
                    tps = prol.tile([128, 128], BF16, name="tps", tag="tps")
                    nc.vector.tensor_copy(tps[:], tp[:])
                    nc.tensor.matmul(pmp[:], w_mem[:, ec * 128:(ec + 1) * 128],
                                     tps[:], start=(ec == 0), stop=(ec == 3))
                nc.vector.tensor_copy(
                    pmT[:, b * T + tcn * 128: b * T + tcn * 128 + 128], pmp[:])

    # ---- prologue: prenet + att pre-gates -> PREG dram [R, G] ----
    preg_dram = dr.tile([R, G], FP32, name="preg_dram")
    with tc.tile_pool(name="pre", bufs=3) as pp:
        wp1 = pp.tile([NMEL, PRE], FP32, name="wp1", tag="wp1", bufs=1)
        nc.sync.dma_start(wp1[:], din["w_pre1_T"].ap())
        wp2 = pp.tile([128, 2 * PRE], FP32, name="wp2", tag="wp2", bufs=1)
        nc.sync.dma_start(
            wp2[:].rearrange("p (k n) -> p k n", k=2),
            din["w_pre2_T"].ap().rearrange("(k p) n -> p k n", p=128))
        wpa = pp.tile([128, 2 * G], FP32, name="wpa", tag="wpa", bufs=1)
        nc.sync.dma_start(
            wpa[:].rearrange("p (k n) -> p k n", k=2),
            din["att_pre_T"].ap().rearrange("(k p) n -> p k n", p=128))
        attb = pp.tile([128, G], FP32, name="attb", tag="attb", bufs=1)
        nc.sync.dma_start(attb[:], din["att_bias"].ap())

        off = 0
        for rbs in rb_sizes:
            pin_blk = pp.tile([NMEL, 512], FP32, name="pin_blk", tag="pin")
            nc.sync.dma_start(pin_blk[:, :rbs], din["pinT"].ap()[:, off:off + rbs])
            h1 = pp.tile([128, 2 * 512], FP32, name="h1", tag="h1", bufs=1)
            for mc in range(2):
                p1 = pat_pool.tile([128, 512], FP32, name="p1", tag="pat")
                nc.tensor.matmul(p1[:, :rbs], wp1[:, mc * 128:(mc + 1) * 128],
                                 pin_blk[:, :rbs], start=True, stop=True)
                nc.scalar.activation(h1[:, mc * 512:mc * 512 + rbs],
                                     p1[:, :rbs], AFT.Relu)
            h2 = pp.tile([128, 2 * 512], FP32, name="h2", tag="h2", bufs=1)
            for mc in range(2):
                p2 = pat_pool.tile([128, 512], FP32, name="p2", tag="pat")
                for kc in range(2):
                    nc.tensor.matmul(
                        p2[:, :rbs],
                        wp2[:, (kc * 2 + mc) * 128:(kc * 2 + mc) * 128 + 128],
                        h1[:, kc * 512:kc * 512 + rbs],
                        start=(kc == 0), stop=(kc == 1))
                nc.scalar.activation(h2[:, mc * 512:mc * 512 + rbs],
                                     p2[:, :rbs], AFT.Relu)
            for mr in range((rbs + 127) // 128):
                mrs = min(128, rbs - mr * 128)
                p3 = pat_pool.tile([128, G], FP32, name="p3", tag="pat")
                for kc in range(2):
                    nc.tensor.matmul(
                        p3[:mrs, :],
                        h2[:, kc * 512 + mr * 128: kc * 512 + mr * 128 + mrs],
                        wpa[:, kc * G:(kc + 1) * G],
                        start=(kc == 0), stop=(kc == 1))
                pr = pp.tile([128, G], FP32, name="pr", tag="pr")
                nc.vector.tensor_tensor(pr[:mrs, :], p3[:mrs, :],
                                        attb[:mrs, :], op=ALU.add)
                nc.sync.dma_start(
                    preg_dram[off + mr * 128: off + mr * 128 + mrs, :],
                    pr[:mrs, :])
            off += rbs

    # ---- state ----
    ahT = st.tile([128, 8 * B], BF16)
    dhT = st.tile([128, 8 * B], BF16)
    ctxT = st.tile([128, 4 * B], BF16)
    c_att = st.tile([B, HL], FP32)
    c_dec = st.tile([B, HL], FP32)
    aw_w = [st.tile([BL, TP], BF16, name=f"aw_w{p}") for p in range(2)]
    aw_c = [st.tile([BL, TP], BF16, name=f"aw_c{p}") for p in range(2)]
    awcum = st.tile([BL, T], FP32, name="awcum")
    dhT_own = st.tile([128, B], BF16)
    wmB = st.tile([16, T], BF16, name="wmB")
    nc.vector.memset(wmB[:], 0.0)
    for t_ in (ahT, dhT, ctxT, c_att, c_dec, aw_w[0], aw_c[0], aw_w[1],
               aw_c[1], dhT_own, awcum):
        nc.vector.memset(t_[:], 0.0)

    rg = [list(range(NC))]

    def lstm_block(pref, gin, gin2, c_state, bias_row):
        """gates activations; returns h_own [B, HL] sbuf tile.

        gin: psum gates or None; gin2: sbuf addend (preg) or None;
        bias_row: [1, G] sbuf row or None."""
        if gin is None:
            gsb = gin2
        else:
            gsb = sb2.tile([B, G], FP32, name=f"{pref}_gsb", tag=f"{pref}_gsb")
            if gin2 is not None:
                nc.vector.tensor_tensor(gsb[:], gin[:], gin2[:], op=ALU.add)
            else:
                nc.vector.tensor_tensor(gsb[:], gin[:], bias_row[:],
                                        op=ALU.add)
        sig = sb2.tile([B, 3 * HL], FP32, name=f"{pref}_sig", tag=f"{pref}_sig")
        nc.scalar.activation(sig[:], gsb[:, 0:3 * HL], AFT.Tanh, scale=0.5)
        nc.vector.tensor_scalar(sig[:], sig[:], 0.5, 0.5, op0=ALU.mult,
                                op1=ALU.add)
        gt = sb2.tile([B, HL], FP32, name=f"{pref}_gt", tag=f"{pref}_gt")
        nc.scalar.activation(gt[:], gsb[:, 3 * HL:4 * HL], AFT.Tanh)
        fc = sb2.tile([B, HL], FP32, name=f"{pref}_fc", tag=f"{pref}_fc")
        nc.vector.tensor_tensor(fc[:], sig[:, HL:2 * HL], c_state[:],
                                op=ALU.mult)
        ig = sb2.tile([B, HL], FP32, name=f"{pref}_ig", tag=f"{pref}_ig")
        nc.vector.tensor_tensor(ig[:], sig[:, 0:HL], gt[:], op=ALU.mult)
        nc.vector.tensor_tensor(c_state[:], fc[:], ig[:], op=ALU.add)
        tch = sb2.tile([B, HL], FP32, name=f"{pref}_tch", tag=f"{pref}_tch")
        nc.scalar.activation(tch[:], c_state[:], AFT.Tanh)
        h = sb2.tile([B, HL], FP32, name=f"{pref}_h", tag=f"{pref}_h")
        nc.vector.tensor_tensor(h[:], sig[:, 2 * HL:3 * HL], tch[:],
                                op=ALU.mult)
        return h

    def kslice(w, k):
        return w[:, k * G:(k + 1) * G] if False else None

    def emit_melgate(t_idx):
        pmg = ps1_pool.tile([B, 81], FP32, name="pmg", tag="ps1")
        for k in range(12):
            lhs = dhT[:, k * B:(k + 1) * B] if k < 8 else \
                ctxT[:, (k - 8) * B:(k - 7) * B]
            nc.tensor.matmul(pmg[:], lhs, w_pg[:, k * 81:(k + 1) * 81],
                             start=(k == 0), stop=(k == 11))
        mg = sb2.tile([B, 81], FP32, name="mg", tag="mg")
        nc.vector.tensor_tensor(mg[:], pmg[:], pgb[:], op=ALU.add)
        nc.sync.dma_start(melgate_out.ap()[t_idx, :, :], mg[:])

    for t in range(td):
        par, pprev = t % 2, 1 - (t % 2)

        preg_sb = sb3.tile([B, G], FP32, name="preg_sb", tag="preg")
        nc.sync.dma_start(preg_sb[:], preg_dram[t * B:(t + 1) * B, :])

        # -- att gates + LSTM --
        if t == 0:
            h_att = lstm_block("ga", None, preg_sb, c_att, None)
        else:
            g_att = pg_pool.tile([B, G], FP32, name="g_att", tag="pg")
            i = 0
            for wt, xt, nk in ((w_att_ctx, ctxT, 4), (w_att_h, ahT, 8)):
                for k in range(nk):
                    nc.tensor.matmul(g_att[:], xt[:, k * B:(k + 1) * B],
                                     wt[:, k * G:(k + 1) * G],
                                     start=(i == 0), stop=(i == 11))
                    i += 1
            h_att = lstm_block("ga", g_att, preg_sb, c_att, None)

        # -- AG1: [dhT(t-1); ahT(t)] --
        tp_ah = pt_pool.tile([128, B], FP32, name="tp_ah", tag="tr")
        nc.tensor.transpose(tp_ah[:], h_att[:], ident[0:B, 0:B])
        ahT_own = sb2.tile([128, B], BF16, name="ahT_own", tag="ahT_own")
        nc.vector.tensor_copy(ahT_own[:], tp_ah[:])

        cin1 = drc.tile([256, B], BF16, name="cin1", tag="cin1")
        cout1 = drc.tile([256 * NC, B], BF16, name="cout1", tag="cout1")
        nc.sync.dma_start(cin1[0:128, :], dhT_own[:])
        nc.sync.dma_start(cin1[128:256, :], ahT_own[:])
        nc.gpsimd.collective_compute(
            "AllGather", ALU.bypass, ins=[cin1.opt()], outs=[cout1.opt()],
            replica_groups=rg)
        nc.sync.dma_start(
            dhT[:].rearrange("p (k b) -> p k b", k=8),
            cout1[:].rearrange("(k two p) b -> p k two b", two=2, p=128)
            [:, :, 0, :])
        nc.sync.dma_start(
            ahT[:].rearrange("p (k b) -> p k b", k=8),
            cout1[:].rearrange("(k two p) b -> p k two b", two=2, p=128)
            [:, :, 1, :])

        # -- melgate for previous step (needs gathered dhT(t-1)) --
        if t > 0:
            emit_melgate(t - 1)

        # -- location features (use aw[t-1]; overlap with AG1) --
        if t > 0:
            col = sb1.tile([62, BL * T], BF16, name="col", tag="col")
            for b in range(BL):
                for ch, srct in ((0, aw_w[pprev]), (1, aw_c[pprev])):
                    nc.sync.dma_start(
                        col[31 * ch:31 * ch + 31, b * T:(b + 1) * T],
                        strided(srct[b:b + 1, 0:T], [[TP, 1], [1, 31], [1, T]]))

        # -- q for own examples --
        pqr = ps1_pool.tile([B, 128], FP32, name="pqr", tag="ps1")
        for k in range(8):
            nc.tensor.matmul(pqr[:], ahT[:, k * B:(k + 1) * B],
                             w_q[:, k * 128:(k + 1) * 128],
                             start=(k == 0), stop=(k == 7))
        qrows = sb2.tile([B, 128], FP32, name="qrows", tag="qrows")
        nc.vector.tensor_copy(qrows[:], pqr[:])
        pqo = ps1_pool.tile([128, BL], FP32, name="pqo", tag="ps1")
        nc.tensor.matmul(pqo[:], qrows[:], qsel[:], start=True, stop=True)
        qTo = sb2.tile([128, BL], FP32, name="qTo", tag="qTo")
        nc.vector.tensor_copy(qTo[:], pqo[:])

        # -- S = tanh(loc + pm + q); scores --
        tanhS = sb1.tile([128, BL * T], BF16, name="tanhS", tag="tanhS")
        for b in range(BL):
            if t == 0:
                nc.scalar.activation(tanhS[:, b * T:(b + 1) * T],
                                     pmT[:, b * T:(b + 1) * T], AFT.Tanh,
                                     bias=qTo[:, b:b + 1])
            else:
                pd = pat_pool.tile([128, T], FP32, name="pd", tag="pat")
                nc.tensor.matmul(pd[:], wloc[:], col[:, b * T:(b + 1) * T],
                                 start=True, stop=False)
                nc.tensor.matmul(pd[:], ident_bf[:],
                                 pmT[:, b * T:(b + 1) * T],
                                 start=False, stop=True)
                nc.scalar.activation(tanhS[:, b * T:(b + 1) * T], pd[:],
                                     AFT.Tanh, bias=qTo[:, b:b + 1])
        psc = ps1_pool.tile([BL, T], FP32, name="psc", tag="ps1")
        for b in range(BL):
            nc.tensor.matmul(psc[:], wsc[:, b * 4:b * 4 + 4],
                             tanhS[:, b * T:(b + 1) * T],
                             start=(b == 0), stop=(b == BL - 1))
        scores = sb1.tile([BL, T], FP32, name="scores", tag="scores")
        nc.vector.tensor_tensor(scores[:], psc[:], mkb[:], op=ALU.add)

        # -- softmax (scores are tanh-bounded: no max subtraction needed) --
        ex_ = sb1.tile([BL, T], FP32, name="ex_", tag="ex_")
        nc.scalar.activation(ex_[:], scores[:], AFT.Exp)
        ssum = sb2.tile([BL, 1], FP32, name="ssum", tag="ssum")
        nc.vector.reduce_sum(ssum[:], ex_[:], AXL.X)
        rinv = sb2.tile([BL, 1], FP32, name="rinv", tag="rinv")
        nc.vector.reciprocal(rinv[:], ssum[:])
        wat = sb1.tile([BL, T], FP32, name="wat", tag="wat")
        nc.vector.tensor_scalar_mul(wat[:], ex_[:], rinv[:])

        # -- alignment output + aw state update (parity par) --
        nc.sync.dma_start(align_out.ap()[t, :, :], wat[:])
        nc.vector.tensor_copy(aw_w[par][:, PAD:PAD + T], wat[:])
        nc.vector.tensor_tensor(awcum[:], awcum[:], wat[:], op=ALU.add)
        nc.vector.tensor_copy(aw_c[par][:, PAD:PAD + T], awcum[:])

        # -- ctx via masked wT (accumulate all 4 examples in one psum) --
        for b in range(BL):
            nc.sync.dma_start(wmB[b * 4 + b:b * 4 + b + 1, :],
                              aw_w[par][b:b + 1, PAD:PAD + T])
        wTm = sb2.tile([128, 4 * 16], BF16, name="wTm", tag="wTm")
        for tcn in range(4):
            ptw = pt_pool.tile([128, 16], BF16, name="ptw", tag="tr")
            nc.tensor.transpose(ptw[:], wmB[:, tcn * 128:(tcn + 1) * 128],
                                ident_bf[0:16, 0:16])
            nc.vector.tensor_copy(wTm[:, tcn * 16:(tcn + 1) * 16], ptw[:])
        pctx = pat_pool.tile([BL, E], FP32, name="pctx", tag="pat")
        i = 0
        for b in range(BL):
            for tcn in range(4):
                nc.tensor.matmul(
                    pctx[:], wTm[:, tcn * 16 + b * 4: tcn * 16 + b * 4 + 4],
                    mem[:, (b * 4 + tcn) * 512:(b * 4 + tcn) * 512 + 512],
                    start=(i == 0), stop=(i == 15))
                i += 1

        # -- AG2: ctx --
        ctx_sb = sb1.tile([BL, E], FP32, name="ctx_sb", tag="ctx_sb")
        nc.vector.tensor_copy(ctx_sb[:], pctx[:])
        cin2 = drc.tile([BL, E], FP32, name="cin2", tag="cin2")
        cout2 = drc.tile([B, E], FP32, name="cout2", tag="cout2")
        nc.sync.dma_start(cin2[:], ctx_sb[:])
        nc.gpsimd.collective_compute(
            "AllGather", ALU.bypass, ins=[cin2.opt()], outs=[cout2.opt()],
            replica_groups=rg)
        ctxr = sb1.tile([B, E], FP32, name="ctxr", tag="ctxr")
        nc.sync.dma_start(ctxr[:], cout2[:])
        for ec in range(4):
            ptc = pt_pool.tile([128, B], FP32, name="ptc", tag="tr")
            nc.tensor.transpose(ptc[:], ctxr[:, ec * 128:(ec + 1) * 128],
                                ident[0:B, 0:B])
            nc.vector.tensor_copy(ctxT[:, ec * B:(ec + 1) * B], ptc[:])

        # -- dec gates + LSTM --
        g_dec = pg_pool.tile([B, G], FP32, name="g_dec", tag="pg")
        parts = [(w_dec_ah, ahT, 8), (w_dec_ctx, ctxT, 4)]
        if t > 0:
            parts.append((w_dec_h, dhT, 8))
        ntot = sum(p[2] for p in parts)
        i = 0
        for wt, xt, nk in parts:
            for k in range(nk):
                nc.tensor.matmul(g_dec[:], xt[:, k * B:(k + 1) * B],
                                 wt[:, k * G:(k + 1) * G],
                                 start=(i == 0), stop=(i == ntot - 1))
                i += 1
        h_dec = lstm_block("gd", g_dec, None, c_dec, decb)

        tp_dh = pt_pool.tile([128, B], FP32, name="tp_dh", tag="tr")
        nc.tensor.transpose(tp_dh[:], h_dec[:], ident[0:B, 0:B])
        nc.vector.tensor_copy(dhT_own[:], tp_dh[:])

    # final gather for dh(td-1) + last melgate
    cin1 = drc.tile([256, B], BF16, name="cin1f", tag="cin1")
    cout1 = drc.tile([256 * NC, B], BF16, name="cout1f", tag="cout1")
    nc.sync.dma_start(cin1[0:128, :], dhT_own[:])
    nc.sync.dma_start(cin1[128:256, :], dhT_own[:])
    nc.gpsimd.collective_compute(
        "AllGather", ALU.bypass, ins=[cin1.opt()], outs=[cout1.opt()],
        replica_groups=rg)
    nc.sync.dma_start(
        dhT[:].rearrange("p (k b) -> p k b", k=8),
        cout1[:].rearrange("(k two p) b -> p k two b", two=2, p=128)
        [:, :, 0, :])
    emit_melgate(td - 1)
    stack.close()


# ======================================================================
# Self-contained entry point: kernel(**inputs) -> (mel, gate, align)
# ======================================================================
from concourse.bass_utils import run_bass_kernel_spmd

TDEC = 400
_CACHE = {}


def kernel(**inputs):
    if "nc" not in _CACHE:
        _CACHE["nc"] = build(TDEC)
    nc = _CACHE["nc"]
    in_maps = host_inputs(inputs, TDEC)
    res = run_bass_kernel_spmd(nc, in_maps, core_ids=list(range(NC)))
    mel, gate, align = postprocess(res.results, TDEC)
    return mel, gate, align
